# revision 14
# baseline (speedup 1.0000x reference)
"""nn_CollaborativeWaterfallMoE — Trainium2 Bass kernel (8 NeuronCores).

Expert parallelism: the cheap 1x1 scorer + sequential waterfall routing run on
the host; each expert's capacity-256 token batch is split in half across two
NeuronCores (8 cores = 4 experts x 2 slices of 128 tokens). Each core runs the
expert conv stack (BN folded into conv weights, projection+classifier folded
into one 256->10 fc) as shifted matmuls in bf16 with fp32 PSUM accumulation.
"""

import math
from contextlib import ExitStack

import numpy as np
import ml_dtypes

E = 4
B = 1024
H = W = 64
CIN = 3
NCLS = 10
TEMP = 0.1
CAP = math.ceil(B / E)  # 256
NI = CAP // 2           # 128 images per core
G = 4                   # images per on-chip group

SD = 66 * 66
SD3 = 34 * 34
SP4 = 18 * 18

_BF16 = ml_dtypes.bfloat16


# ---------------------------------------------------------------------------
# Environment fixup: this walrus build accepts only ONE sync-wait per CTRL
# instruction, but Tile's kernel-tail drain accumulates one wait per live
# semaphore. Split the waits over a chain of nops.
# ---------------------------------------------------------------------------

def _install_drain_patch():
    import concourse.mybir as mybir
    import concourse.tile as tile_mod
    from concourse.vector_clock import ScopedClock

    if getattr(tile_mod.TileContext, "_drain_patch_installed", False):
        return

    def _patched(self, tick_clock, wait_clock):
        probe = self.nc.sync.nop(nofuse=True, hint="tail_wait_probe")
        wait_clock.add_sem_waits(
            probe.ins, ScopedClock({None: tick_clock.global_clock}))
        si = probe.ins.sync_info
        waits = list(si.on_wait) if si is not None else []
        probe.ins.sync_info = mybir.SyncInfo(
            on_wait=waits[:1], on_update=list(si.on_update) if si else [])
        for w in waits[1:]:
            n = self.nc.sync.nop(nofuse=True, hint="tail_wait_chain")
            n.ins.sync_info = mybir.SyncInfo(on_wait=[w], on_update=[])
        self.nc.sync.drain()
        self.nc.all_engine_barrier()
        assert self.sems is not None
        popped = self.nc._tile_sem_poison_stack.pop()
        assert popped is self._sem_poison
        self.nc.clear_and_free_semaphores(list(self.sems.allocated().values()))
        self.nc.all_engine_barrier()

    tile_mod.TileContext._drain_and_barrier = _patched
    tile_mod.TileContext._drain_patch_installed = True


def _split_multi_waits(nc):
    """This walrus build accepts at most one sync-wait per instruction;
    hoist extra waits onto nops inserted just before, on the same engine."""
    import concourse.mybir as mybir

    ctr = 0
    for f in nc.m.functions:
        for bb in f.blocks:
            new = []
            changed = False
            for inst in bb.instructions:
                si = getattr(inst, "sync_info", None)
                if si is not None and si.on_wait is not None \
                        and len(si.on_wait) > 1:
                    waits = list(si.on_wait)
                    for w in waits[:-1]:
                        ctr += 1
                        nop = mybir.InstNoOp(
                            name=f"I-waitsplit-{ctr}", ins=[], outs=[])
                        nop.engine = inst.engine
                        nop.sync_info = mybir.SyncInfo(
                            on_wait=[w], on_update=[])
                        new.append(nop)
                    inst.sync_info = mybir.SyncInfo(
                        on_wait=[waits[-1]],
                        on_update=list(si.on_update or []))
                    changed = True
                new.append(inst)
            if changed:
                bb.instructions = new
    return ctr


# ---------------------------------------------------------------------------
# Device kernel builder (one expert's conv stack over NI images)
# ---------------------------------------------------------------------------

def _build_nc(split_waits=True):
    import concourse.bass as bass
    import concourse.mybir as mybir
    import concourse.tile as tile

    F32 = mybir.dt.float32
    BF16 = mybir.dt.bfloat16
    AF = mybir.ActivationFunctionType
    ALU = mybir.AluOpType

    NG = NI // G
    nc = bass.Bass()

    b27d = nc.declare_dram_parameter("b27", [128, NI * 1024], BF16, isOutput=False)
    w1d = nc.declare_dram_parameter("w1s", [128, 64], BF16, isOutput=False)
    wst2d = nc.declare_dram_parameter("wst2", [128, 192], BF16, isOutput=False)
    ws2d = nc.declare_dram_parameter("ws2", [64, 192], BF16, isOutput=False)
    wst3d = nc.declare_dram_parameter("wst3", [128, 384], BF16, isOutput=False)
    ws3d = nc.declare_dram_parameter("ws3", [64, 384], BF16, isOutput=False)
    w4d = nc.declare_dram_parameter("w4", [128, 18 * 128], BF16, isOutput=False)
    wfcd = nc.declare_dram_parameter("wfc", [128, 20], F32, isOutput=False)
    bcd = nc.declare_dram_parameter("bconv", [128, 8], F32, isOutput=False)
    bfcd = nc.declare_dram_parameter("bfc", [10, 1], F32, isOutput=False)
    outd = nc.declare_dram_parameter("out", [10, NI], F32, isOutput=True)

    with tile.TileContext(nc) as tc:
        with ExitStack() as ctx:
            persist = ctx.enter_context(tc.tile_pool(name="persist", bufs=1))
            const = ctx.enter_context(tc.tile_pool(name="const", bufs=1))
            io = ctx.enter_context(tc.tile_pool(name="io", bufs=2))
            scr = ctx.enter_context(tc.tile_pool(name="scr", bufs=3))
            psum = ctx.enter_context(tc.tile_pool(name="psum", bufs=4, space="PSUM"))

            # double-buffered padded input buffers (manual 2-slot rotation)
            Db = [persist.tile([128, G * SD], BF16, tag=f"D{i}", name=f"D{i}")
                  for i in range(2)]
            D3b = [persist.tile([128, G * SD3], BF16, tag=f"D3{i}", name=f"D3{i}")
                   for i in range(2)]
            X4b = [persist.tile([128, G * SP4], BF16, tag=f"X4{i}", name=f"X4{i}")
                   for i in range(2)]
            zlo = persist.tile([128, NI], F32, tag="zlo")
            zhi = persist.tile([128, NI], F32, tag="zhi")
            outb = persist.tile([10, NI], F32, tag="outb")

            w1 = const.tile([128, 64], BF16, tag="w1")
            wst2 = const.tile([128, 192], BF16, tag="wst2")
            ws2 = const.tile([64, 192], BF16, tag="ws2")
            wst3 = const.tile([128, 384], BF16, tag="wst3")
            ws3 = const.tile([64, 384], BF16, tag="ws3")
            w4 = const.tile([128, 18 * 128], BF16, tag="w4")
            wfc = const.tile([128, 20], F32, tag="wfc")
            bc = const.tile([128, 8], F32, tag="bc")
            bfc = const.tile([10, 1], F32, tag="bfc")

            for t, d in ((w1, w1d), (wst2, wst2d), (ws2, ws2d), (wst3, wst3d),
                         (ws3, ws3d), (w4, w4d), (wfc, wfcd), (bc, bcd),
                         (bfc, bfcd)):
                nc.sync.dma_start(out=t[:, :], in_=d[:, :])

            for t in Db + D3b + X4b:
                nc.gpsimd.memset(t[:, :], 0.0)

            def stage1(gi):
                """B27 DMA + L1 matmuls + evict to D + per-pair mirror DMAs."""
                D = Db[gi % 2]
                Dv = D.rearrange("p (g r s) -> p g r s", g=G, r=66, s=66)
                b27 = io.tile([128, G * 1024], BF16, tag="b27")
                nc.sync.dma_start(
                    out=b27[:, :],
                    in_=b27d[:, gi * G * 1024:(gi + 1) * G * 1024])
                b27v = b27.rearrange("p (g c) -> p g c", g=G)

                for q in range(G // 2):
                    ge, go = 2 * q, 2 * q + 1
                    for s in range(4):
                        ps = psum.tile([128, 1024], F32, tag="mm")
                        bp = 32 * s
                        for h in range(2):
                            for par, gl in ((0, ge), (64, go)):
                                nc.tensor.matmul(
                                    ps[par:par + 64, h * 512:(h + 1) * 512],
                                    w1[bp:bp + 27, 0:64],
                                    b27v[bp:bp + 27, gl, h * 512:(h + 1) * 512],
                                    start=True, stop=True,
                                    tile_position=(bp, par))
                        R = 16 * s
                        psv = ps.rearrange("p (r c) -> p r c", r=16)
                        nc.scalar.activation(
                            Dv[0:64, ge, R + 1:R + 17, 1:65], psv[0:64, :, :],
                            AF.Relu, bias=bc[0:64, 0:1])
                        nc.vector.tensor_scalar(
                            Dv[64:128, go, R:R + 16, 1:65], psv[64:128, :, :],
                            bc[64:128, 0:1], 0.0, op0=ALU.add, op1=ALU.max)
                    # per-pair mirror: fill odd image lower, even image upper
                    nc.sync.dma_start(
                        out=D[0:64, go * SD + 66:go * SD + SD],
                        in_=D[64:128, go * SD:go * SD + SD - 66])
                    nc.sync.dma_start(
                        out=D[64:128, ge * SD:ge * SD + SD - 66],
                        in_=D[0:64, ge * SD + 66:ge * SD + SD])

            def stage234(gi):
                D = Db[gi % 2]
                D3 = D3b[gi % 2]
                X4 = X4b[gi % 2]
                Dv = D.rearrange("p (g r s) -> p g r s", g=G, r=66, s=66)
                D3v = D3.rearrange("p (g r s) -> p g r s", g=G, r=34, s=34)
                X4v = X4.rearrange("p (g r s) -> p g r s", g=G, r=18, s=18)

                # ---- L2: M-packed shifted matmuls ----
                # stationary [W_kj0 | W_kj1]: psum lower = kj0(+kj2+edge)
                # contribs at pixel j, upper = kj1 contribs at pixel j-1.
                # combine: y[j] = lo[j] + up[j+1] (+ edge matmuls for j=63).
                for gl in range(G):
                    for v in range(2):  # 4-chunk pass, rows 32v..32v+32
                        pst = [psum.tile([128, 1024], F32, tag="mm",
                                         name=f"ps2_{gl}_{v}_{i}")
                               for i in range(2)]
                        for w_idx, (lhs, K, cb, M, stop) in enumerate((
                                (wst2[:, 0:128], 128, 0, 128, False),
                                (wst2[:, 128:192], 128, 2, 64, False),
                                (ws2[:, 128:192], 64, 2, 64, False))):
                            for c in range(4):
                                R = 32 * v + 8 * c
                                ps = pst[c // 2]
                                bk = (c % 2) * 512
                                if K == 128:
                                    mv = Dv[0:128, gl, R:R + 8, cb:cb + 64]
                                else:
                                    mv = Dv[0:64, gl, R + 2:R + 10, cb:cb + 64]
                                nc.tensor.matmul(
                                    ps[0:M, bk:bk + 512], lhs, mv,
                                    start=(w_idx == 0), stop=stop)
                        for eidx, (lhs, K) in enumerate((
                                (wst2[:, 64:128], 128), (ws2[:, 64:128], 64))):
                            for c in range(4):
                                R = 32 * v + 8 * c
                                ps = pst[c // 2]
                                psv = ps.rearrange("p (b r c) -> p b r c",
                                                   b=2, r=8)
                                if K == 128:
                                    mv = Dv[0:128, gl, R:R + 8, 64:65]
                                else:
                                    mv = Dv[0:64, gl, R + 2:R + 10, 64:65]
                                nc.tensor.matmul(
                                    psv[0:64, c % 2, :, 63:64], lhs, mv,
                                    start=False, stop=False)
                        # last: ki2/kj01 (M=128) carries the group stop so the
                        # full 128-partition region closes before eviction
                        for c in range(4):
                            R = 32 * v + 8 * c
                            ps = pst[c // 2]
                            bk = (c % 2) * 512
                            nc.tensor.matmul(
                                ps[0:128, bk:bk + 512], ws2[:, 0:128],
                                Dv[0:64, gl, R + 2:R + 10, 0:64],
                                start=False, stop=True)
                        for c in range(4):
                            R = 32 * v + 8 * c
                            ps = pst[c // 2]
                            psv = ps.rearrange("p (b r c) -> p b r c",
                                               b=2, r=8)
                            bi = c % 2
                            upr = scr.tile([128, 512], BF16, tag="upr")
                            nc.scalar.activation(
                                upr[64:128, :],
                                ps[64:128, bi * 512:(bi + 1) * 512], AF.Copy)
                            upx = scr.tile([128, 512], BF16, tag="upx")
                            nc.sync.dma_start(out=upx[0:64, :],
                                              in_=upr[64:128, :])
                            upv = upx.rearrange("p (r c) -> p r c", r=8)
                            t0 = scr.tile([128, 512], BF16, tag="t0")
                            t0v = t0.rearrange("p (r c) -> p r c", r=8)
                            nc.vector.scalar_tensor_tensor(
                                t0v[0:64, :, 0:63], psv[0:64, bi, :, 0:63],
                                bc[0:64, 1:2], upv[0:64, :, 1:64],
                                op0=ALU.add, op1=ALU.add)
                            nc.vector.tensor_scalar(
                                t0v[0:64, :, 63:64], psv[0:64, bi, :, 63:64],
                                bc[0:64, 1:2], None, op0=ALU.add)
                            t0w = t0.rearrange("p (r c w) -> p r c w",
                                               r=8, c=32)
                            th = scr.tile([128, 256], BF16, tag="th")
                            thv = th.rearrange("p (r c) -> p r c", r=8)
                            nc.vector.tensor_tensor(
                                thv[0:64, :, :], t0w[0:64, :, :, 0],
                                t0w[0:64, :, :, 1], op=ALU.max)
                            PR = 16 * v + 4 * c
                            thv2 = th.rearrange("p (r w c) -> p r w c",
                                                r=4, w=2)
                            nc.vector.scalar_tensor_tensor(
                                D3v[0:64, gl, PR + 1:PR + 5, 1:33],
                                thv2[0:64, :, 0, :], 0.0,
                                thv2[0:64, :, 1, :],
                                op0=ALU.max, op1=ALU.max)
                for q in range(G // 2):
                    nc.sync.dma_start(
                        out=D3[64:128,
                               2 * q * SD3:(2 * q + 2) * SD3 - 34],
                        in_=D3[0:64,
                               2 * q * SD3 + 34:(2 * q + 2) * SD3])

                # ---- L3 ----
                for gl in range(G):
                    ps = psum.tile([128, 1024], F32, tag="mm")
                    for kj in range(3):
                        for h in range(2):
                            nc.tensor.matmul(
                                ps[0:128, h * 512:(h + 1) * 512],
                                wst3[:, kj * 128:(kj + 1) * 128],
                                D3v[0:128, gl, 16 * h:16 * h + 16, kj:kj + 32],
                                start=(kj == 0), stop=False)
                    for kj in range(3):
                        for h in range(2):
                            nc.tensor.matmul(
                                ps[0:128, h * 512:(h + 1) * 512],
                                ws3[:, kj * 128:(kj + 1) * 128],
                                D3v[0:64, gl, 16 * h + 2:16 * h + 18, kj:kj + 32],
                                start=False, stop=(kj == 2))
                    t3 = scr.tile([128, 1024], BF16, tag="t3")
                    nc.scalar.activation(
                        t3[:, :], ps[:, :], AF.Relu, bias=bc[:, 2:3])
                    t3v = t3.rearrange("p (r c w) -> p r c w", r=32, c=16)
                    th3 = scr.tile([128, 512], BF16, tag="th3")
                    th3v = th3.rearrange("p (r c) -> p r c", r=32)
                    nc.vector.tensor_tensor(
                        th3v[:, :, :], t3v[:, :, :, 0], t3v[:, :, :, 1],
                        op=ALU.max)
                    th3v2 = th3.rearrange("p (r w c) -> p r w c", r=16, w=2)
                    nc.vector.tensor_tensor(
                        X4v[0:128, gl, 1:17, 1:17],
                        th3v2[:, :, 0, :], th3v2[:, :, 1, :], op=ALU.max)

                # ---- L4 (pairs share each weight via 2 live psum tiles) ----
                for f in range(2):
                    pss = [psum.tile([128, 1024], F32, tag="mm",
                                     name=f"ps4_{f}_{i}")
                           for i in range(G // 2)]
                    for ki in range(3):
                        for kj in range(3):
                            m = (ki * 3 + kj) * 2 + f
                            for q in range(G // 2):
                                ge = 2 * q
                                nc.tensor.matmul(
                                    pss[q][0:128, 0:512],
                                    w4[:, m * 128:(m + 1) * 128],
                                    X4v[0:128, ge:ge + 2, ki:ki + 16,
                                        kj:kj + 16],
                                    start=(ki == 0 and kj == 0),
                                    stop=(ki == 2 and kj == 2))
                    zt = zlo if f == 0 else zhi
                    for q in range(G // 2):
                        l4o = scr.tile([128, 512], BF16, tag="l4o")
                        for u in range(2):
                            img = gi * G + 2 * q + u
                            nc.scalar.activation(
                                l4o[:, u * 256:(u + 1) * 256],
                                pss[q][:, u * 256:(u + 1) * 256],
                                AF.Relu, bias=bc[:, 3 + f:4 + f],
                                accum_out=zt[:, img:img + 1])

            # one-group software pipeline: PE never waits on the evict/pool
            # chain of the group it just produced.
            for gi in range(NG):
                stage1(gi)
                if gi > 0:
                    stage234(gi - 1)
            stage234(NG - 1)

            # ---- FC: 256 -> 10 (fp32) ----
            fps = psum.tile([128, 1024], F32, tag="mm", name="fcps")[0:10, 0:NI]
            nc.tensor.matmul(fps[:, :], wfc[:, 0:10], zlo[:, :],
                             start=True, stop=False)
            nc.tensor.matmul(fps[:, :], wfc[:, 10:20], zhi[:, :],
                             start=False, stop=True)
            nc.scalar.activation(outb[:, :], fps[:, :], AF.Identity,
                                 bias=bfc[:, 0:1])
            nc.sync.dma_start(out=outd[:, :], in_=outb[:, :])

    if split_waits:
        _split_multi_waits(nc)
    return nc


# ---------------------------------------------------------------------------
# Host-side prep
# ---------------------------------------------------------------------------

def _fold_bn(w, b, g, be, m, v):
    s = (np.asarray(g, np.float64) / np.sqrt(np.asarray(v, np.float64) + 1e-5))
    wf = np.asarray(w, np.float64) * s[:, None, None, None]
    bf = (np.asarray(b, np.float64) - np.asarray(m, np.float64)) * s \
        + np.asarray(be, np.float64)
    return wf, bf


def _prep_core_inputs(x_shard, p, e):
    w1f, b1f = _fold_bn(p["c1w"][e], p["c1b"][e], p["bn1g"][e], p["bn1b"][e],
                        p["bn1m"][e], p["bn1v"][e])
    w2f, b2f = _fold_bn(p["c2w"][e], p["c2b"][e], p["bn2g"][e], p["bn2b"][e],
                        p["bn2m"][e], p["bn2v"][e])
    w3f, b3f = _fold_bn(p["c3w"][e], p["c3b"][e], p["bn3g"][e], p["bn3b"][e],
                        p["bn3m"][e], p["bn3v"][e])
    w4f, b4f = _fold_bn(p["c4w"][e], p["c4b"][e], p["bn4g"][e], p["bn4b"][e],
                        p["bn4m"][e], p["bn4v"][e])

    xp = np.pad(x_shard, ((0, 0), (0, 0), (1, 1), (1, 1)))
    arr = np.empty((27, NI, 64, 64), np.float32)
    for ki in range(3):
        for kj in range(3):
            for c in range(3):
                arr[(ki * 3 + kj) * 3 + c] = xp[:, c, ki:ki + 64, kj:kj + 64]
    b27 = np.zeros((128, NI, 16, 64), np.float32)
    for b in range(4):
        b27[32 * b:32 * b + 27] = arr[:, :, 16 * b:16 * b + 16, :]
    b27 = b27.reshape(128, NI * 1024).astype(_BF16)

    w1s = np.zeros((128, 64), np.float64)
    w1k = w1f.transpose(2, 3, 1, 0).reshape(27, 64)
    for b in range(4):
        w1s[32 * b:32 * b + 27] = w1k

    wst2 = np.zeros((128, 192), np.float64)
    ws2 = np.zeros((64, 192), np.float64)
    for kj in range(3):
        wst2[0:64, kj * 64:(kj + 1) * 64] = w2f[:, :, 0, kj].T
        wst2[64:128, kj * 64:(kj + 1) * 64] = w2f[:, :, 1, kj].T
        ws2[:, kj * 64:(kj + 1) * 64] = w2f[:, :, 2, kj].T

    wst3 = np.zeros((128, 384), np.float64)
    ws3 = np.zeros((64, 384), np.float64)
    for kj in range(3):
        wst3[0:64, kj * 128:(kj + 1) * 128] = w3f[:, :, 0, kj].T
        wst3[64:128, kj * 128:(kj + 1) * 128] = w3f[:, :, 1, kj].T
        ws3[:, kj * 128:(kj + 1) * 128] = w3f[:, :, 2, kj].T

    w4 = np.zeros((128, 18 * 128), np.float64)
    for ki in range(3):
        for kj in range(3):
            for f in range(2):
                m = (ki * 3 + kj) * 2 + f
                w4[:, m * 128:(m + 1) * 128] = \
                    w4f[128 * f:128 * (f + 1), :, ki, kj].T

    wf = (np.asarray(p["cw"][e], np.float64)
          @ np.asarray(p["pw"][e], np.float64)) / 256.0
    wfc = np.zeros((128, 20), np.float64)
    wfc[:, 0:10] = wf[:, 0:128].T
    wfc[:, 10:20] = wf[:, 128:256].T
    bfc = (np.asarray(p["cw"][e], np.float64)
           @ np.asarray(p["pb"][e], np.float64)
           + np.asarray(p["cb"][e], np.float64)).reshape(10, 1)

    bconv = np.zeros((128, 8), np.float64)
    bconv[0:64, 0] = b1f
    bconv[64:128, 0] = b1f
    bconv[0:64, 1] = b2f
    bconv[64:128, 1] = b2f
    bconv[:, 2] = b3f
    bconv[:, 3] = b4f[0:128]
    bconv[:, 4] = b4f[128:256]

    return {
        "b27": b27,
        "w1s": w1s.astype(_BF16), "wst2": wst2.astype(_BF16),
        "ws2": ws2.astype(_BF16), "wst3": wst3.astype(_BF16),
        "ws3": ws3.astype(_BF16), "w4": w4.astype(_BF16),
        "wfc": wfc.astype(np.float32), "bconv": bconv.astype(np.float32),
        "bfc": bfc.astype(np.float32),
    }


# ---------------------------------------------------------------------------
# Host-side scorer + waterfall routing (faithful to the reference)
# ---------------------------------------------------------------------------

def _scores_noisy(x, sw, sb, slw, slb):
    # scorer per expert: conv1x1 -> relu -> global avg pool -> linear(8->1)
    scores = np.empty((B, E), np.float32)
    xf = np.asarray(x, np.float32)
    for e in range(E):
        w = np.asarray(sw[e], np.float32)[:, :, 0, 0]          # [8,3]
        h = np.einsum("bchw,oc->bohw", xf, w, optimize=True) \
            + np.asarray(sb[e], np.float32)[None, :, None, None]
        h = np.maximum(h, 0.0)
        hm = h.mean(axis=(2, 3), dtype=np.float32)             # [B,8]
        scores[:, e] = hm @ np.asarray(slw[e], np.float32)[0] \
            + np.asarray(slb[e], np.float32)[0]

    # Match the reference bit-for-bit: same jax calls, default device/PRNG
    # (this env defaults to the rbg PRNG, which is backend-dependent).
    import jax
    import jax.numpy as jnp
    u = jax.random.uniform(
        jax.random.key(42), (B, E), jnp.float32, 1e-7, 1.0 - 1e-7)
    gumbel = -jnp.log(-jnp.log(u))
    return np.asarray((jnp.asarray(scores) + gumbel) / TEMP)


def _route(scores_noisy, cap_C):
    s = np.asarray(scores_noisy, np.float64)
    Bn, En = s.shape
    owner = -np.ones(Bn, np.int64)
    cap = np.zeros(En, np.int64)
    remaining = np.arange(Bn)
    it = 0
    while remaining.size > 0:
        st = s[remaining].copy()
        deficit = np.clip(cap / float(cap_C), 0.0, 1.0)
        st = st * (1.0 - deficit)
        full = cap >= cap_C
        st[:, full] = -np.inf
        best = st.argmax(axis=1)
        taken = np.zeros(remaining.size, bool)
        quota = 2 ** it
        for e in range(En):
            want = np.nonzero(best == e)[0]
            if want.size == 0:
                continue
            space = min(cap_C - int(cap[e]), quota)
            if space <= 0:
                continue
            sel = want[:space]
            owner[remaining[sel]] = e
            cap[e] += sel.size
            taken[sel] = True
        remaining = remaining[~taken]
        it += 1
    idx = np.zeros((En, cap_C), np.int64)
    mask = np.zeros((En, cap_C), np.float32)
    for e in range(En):
        ids = np.nonzero(owner == e)[0]
        idx[e, :ids.size] = ids
        mask[e, :ids.size] = 1.0
    return idx, mask


# ---------------------------------------------------------------------------
# Entry point
# ---------------------------------------------------------------------------

_NC_CACHE = {}


def kernel(**inputs):
    from concourse.bass_utils import run_bass_kernel_spmd

    _install_drain_patch()

    p = {k: np.asarray(v) for k, v in inputs.items()}
    x = p["x"].astype(np.float32, copy=False)

    sn = _scores_noisy(x, p["sw"], p["sb"], p["slw"], p["slb"])
    idx, mask = _route(sn, CAP)

    if "nc" not in _NC_CACHE:
        _NC_CACHE["nc"] = _build_nc()
    nc = _NC_CACHE["nc"]

    in_maps = []
    for core in range(8):
        e = core // 2
        half = core % 2
        ids = idx[e, half * NI:(half + 1) * NI]
        in_maps.append(_prep_core_inputs(x[ids], p, e))

    res = run_bass_kernel_spmd(nc, in_maps, list(range(8)))
    outs = res.results

    out = np.zeros((B, NCLS), np.float32)
    for core in range(8):
        e = core // 2
        half = core % 2
        ids = idx[e, half * NI:(half + 1) * NI]
        m = mask[e, half * NI:(half + 1) * NI]
        np.add.at(out, ids, outs[core]["out"].T * m[:, None])
    return out


# revision 15
# speedup vs baseline: 1.5909x; 1.5909x over previous
"""nn_CollaborativeWaterfallMoE — Trainium2 Bass kernel (8 NeuronCores).

Expert parallelism: the cheap 1x1 scorer + sequential waterfall routing run on
the host; each expert's capacity-256 token batch is split in half across two
NeuronCores (8 cores = 4 experts x 2 slices of 128 tokens). Each core runs the
expert conv stack (BN folded into conv weights, projection+classifier folded
into one 256->10 fc) as shifted matmuls in bf16 with fp32 PSUM accumulation.
"""

import math
from contextlib import ExitStack

import numpy as np
import ml_dtypes

E = 4
B = 1024
H = W = 64
CIN = 3
NCLS = 10
TEMP = 0.1
CAP = math.ceil(B / E)  # 256
NI = CAP // 2           # 128 images per core
G = 4                   # images per on-chip group

SD = 66 * 66
SD3 = 34 * 34
SP4 = 18 * 18

_BF16 = ml_dtypes.bfloat16


# ---------------------------------------------------------------------------
# Environment fixup: this walrus build accepts only ONE sync-wait per CTRL
# instruction, but Tile's kernel-tail drain accumulates one wait per live
# semaphore. Split the waits over a chain of nops.
# ---------------------------------------------------------------------------

def _install_drain_patch():
    import concourse.mybir as mybir
    import concourse.tile as tile_mod
    from concourse.vector_clock import ScopedClock

    if getattr(tile_mod.TileContext, "_drain_patch_installed", False):
        return

    def _patched(self, tick_clock, wait_clock):
        probe = self.nc.sync.nop(nofuse=True, hint="tail_wait_probe")
        wait_clock.add_sem_waits(
            probe.ins, ScopedClock({None: tick_clock.global_clock}))
        si = probe.ins.sync_info
        waits = list(si.on_wait) if si is not None else []
        probe.ins.sync_info = mybir.SyncInfo(
            on_wait=waits[:1], on_update=list(si.on_update) if si else [])
        for w in waits[1:]:
            n = self.nc.sync.nop(nofuse=True, hint="tail_wait_chain")
            n.ins.sync_info = mybir.SyncInfo(on_wait=[w], on_update=[])
        self.nc.sync.drain()
        self.nc.all_engine_barrier()
        assert self.sems is not None
        popped = self.nc._tile_sem_poison_stack.pop()
        assert popped is self._sem_poison
        self.nc.clear_and_free_semaphores(list(self.sems.allocated().values()))
        self.nc.all_engine_barrier()

    tile_mod.TileContext._drain_and_barrier = _patched
    tile_mod.TileContext._drain_patch_installed = True


def _split_multi_waits(nc):
    """This walrus build accepts at most one sync-wait per instruction;
    hoist extra waits onto nops inserted just before, on the same engine."""
    import concourse.mybir as mybir

    ctr = 0
    for f in nc.m.functions:
        for bb in f.blocks:
            new = []
            changed = False
            for inst in bb.instructions:
                si = getattr(inst, "sync_info", None)
                if si is not None and si.on_wait is not None \
                        and len(si.on_wait) > 1:
                    waits = list(si.on_wait)
                    for w in waits[:-1]:
                        ctr += 1
                        nop = mybir.InstNoOp(
                            name=f"I-waitsplit-{ctr}", ins=[], outs=[])
                        nop.engine = inst.engine
                        nop.sync_info = mybir.SyncInfo(
                            on_wait=[w], on_update=[])
                        new.append(nop)
                    inst.sync_info = mybir.SyncInfo(
                        on_wait=[waits[-1]],
                        on_update=list(si.on_update or []))
                    changed = True
                new.append(inst)
            if changed:
                bb.instructions = new
    return ctr


# ---------------------------------------------------------------------------
# Device kernel builder (one expert's conv stack over NI images)
# ---------------------------------------------------------------------------

def _build_nc(split_waits=True):
    import concourse.bass as bass
    import concourse.mybir as mybir
    import concourse.tile as tile

    F32 = mybir.dt.float32
    BF16 = mybir.dt.bfloat16
    AF = mybir.ActivationFunctionType
    ALU = mybir.AluOpType

    NG = NI // G
    nc = bass.Bass()

    b27d = nc.declare_dram_parameter("b27", [128, NI * 1024], BF16, isOutput=False)
    w1d = nc.declare_dram_parameter("w1s", [128, 64], BF16, isOutput=False)
    wst2d = nc.declare_dram_parameter("wst2", [128, 192], BF16, isOutput=False)
    ws2d = nc.declare_dram_parameter("ws2", [64, 192], BF16, isOutput=False)
    wst3d = nc.declare_dram_parameter("wst3", [128, 384], BF16, isOutput=False)
    ws3d = nc.declare_dram_parameter("ws3", [64, 384], BF16, isOutput=False)
    w4d = nc.declare_dram_parameter("w4", [128, 18 * 128], BF16, isOutput=False)
    wfcd = nc.declare_dram_parameter("wfc", [128, 20], F32, isOutput=False)
    bcd = nc.declare_dram_parameter("bconv", [128, 8], F32, isOutput=False)
    bfcd = nc.declare_dram_parameter("bfc", [10, 1], F32, isOutput=False)
    outd = nc.declare_dram_parameter("out", [10, NI], F32, isOutput=True)

    with tile.TileContext(nc) as tc:
        with ExitStack() as ctx:
            persist = ctx.enter_context(tc.tile_pool(name="persist", bufs=1))
            const = ctx.enter_context(tc.tile_pool(name="const", bufs=1))
            io = ctx.enter_context(tc.tile_pool(name="io", bufs=2))
            scr = ctx.enter_context(tc.tile_pool(name="scr", bufs=3))
            psum = ctx.enter_context(tc.tile_pool(name="psum", bufs=4, space="PSUM"))

            # double-buffered padded input buffers (manual 2-slot rotation)
            Db = [persist.tile([128, G * SD], BF16, tag=f"D{i}", name=f"D{i}")
                  for i in range(2)]
            D3b = [persist.tile([128, G * SD3], BF16, tag=f"D3{i}", name=f"D3{i}")
                   for i in range(2)]
            X4b = [persist.tile([128, G * SP4], BF16, tag=f"X4{i}", name=f"X4{i}")
                   for i in range(2)]
            zlo = persist.tile([128, NI], F32, tag="zlo")
            zhi = persist.tile([128, NI], F32, tag="zhi")
            outb = persist.tile([10, NI], F32, tag="outb")

            w1 = const.tile([128, 64], BF16, tag="w1")
            wst2 = const.tile([128, 192], BF16, tag="wst2")
            ws2 = const.tile([64, 192], BF16, tag="ws2")
            wst3 = const.tile([128, 384], BF16, tag="wst3")
            ws3 = const.tile([64, 384], BF16, tag="ws3")
            w4 = const.tile([128, 18 * 128], BF16, tag="w4")
            wfc = const.tile([128, 20], F32, tag="wfc")
            bc = const.tile([128, 8], F32, tag="bc")
            bfc = const.tile([10, 1], F32, tag="bfc")

            for t, d in ((w1, w1d), (wst2, wst2d), (ws2, ws2d), (wst3, wst3d),
                         (ws3, ws3d), (w4, w4d), (wfc, wfcd), (bc, bcd),
                         (bfc, bfcd)):
                nc.sync.dma_start(out=t[:, :], in_=d[:, :])

            for t in Db + D3b + X4b:
                nc.gpsimd.memset(t[:, :], 0.0)

            def stage1(gi):
                """B27 DMA + L1 matmuls + evict to D + per-pair mirror DMAs."""
                D = Db[gi % 2]
                Dv = D.rearrange("p (g r s) -> p g r s", g=G, r=66, s=66)
                b27 = io.tile([128, G * 1024], BF16, tag="b27")
                nc.sync.dma_start(
                    out=b27[:, :],
                    in_=b27d[:, gi * G * 1024:(gi + 1) * G * 1024])
                b27v = b27.rearrange("p (g c) -> p g c", g=G)

                for q in range(G // 2):
                    ge, go = 2 * q, 2 * q + 1
                    for s in range(4):
                        ps = psum.tile([128, 1024], F32, tag="mm")
                        bp = 32 * s
                        for h in range(2):
                            for par, gl in ((0, ge), (64, go)):
                                nc.tensor.matmul(
                                    ps[par:par + 64, h * 512:(h + 1) * 512],
                                    w1[bp:bp + 27, 0:64],
                                    b27v[bp:bp + 27, gl, h * 512:(h + 1) * 512],
                                    start=True, stop=True,
                                    tile_position=(bp, par))
                        R = 16 * s
                        psv = ps.rearrange("p (r c) -> p r c", r=16)
                        nc.scalar.activation(
                            Dv[0:64, ge, R + 1:R + 17, 1:65], psv[0:64, :, :],
                            AF.Relu, bias=bc[0:64, 0:1])
                        nc.vector.tensor_scalar(
                            Dv[64:128, go, R:R + 16, 1:65], psv[64:128, :, :],
                            bc[64:128, 0:1], 0.0, op0=ALU.add, op1=ALU.max)
                    # per-pair mirror: fill odd image lower, even image upper
                    nc.sync.dma_start(
                        out=D[0:64, go * SD + 66:go * SD + SD],
                        in_=D[64:128, go * SD:go * SD + SD - 66])
                    nc.sync.dma_start(
                        out=D[64:128, ge * SD:ge * SD + SD - 66],
                        in_=D[0:64, ge * SD + 66:ge * SD + SD])

            def stage234(gi):
                D = Db[gi % 2]
                D3 = D3b[gi % 2]
                X4 = X4b[gi % 2]
                Dv = D.rearrange("p (g r s) -> p g r s", g=G, r=66, s=66)
                D3v = D3.rearrange("p (g r s) -> p g r s", g=G, r=34, s=34)
                X4v = X4.rearrange("p (g r s) -> p g r s", g=G, r=18, s=18)

                # ---- L2: 64->64 shifted matmuls (groups in separate banks) --
                for q in range(G // 2):
                    ge, go = 2 * q, 2 * q + 1
                    for s in range(8):
                        ps = psum.tile([128, 1024], F32, tag="mm")
                        R = 8 * s
                        for kj in range(3):
                            for par, gl in ((0, ge), (64, go)):
                                nc.tensor.matmul(
                                    ps[par:par + 64, (par * 8):(par * 8) + 512],
                                    wst2[:, kj * 64:(kj + 1) * 64],
                                    Dv[0:128, gl, R:R + 8, kj:kj + 64],
                                    start=(kj == 0), stop=False)
                        for kj in range(3):
                            for par, gl in ((0, ge), (64, go)):
                                nc.tensor.matmul(
                                    ps[par:par + 64, (par * 8):(par * 8) + 512],
                                    ws2[:, kj * 64:(kj + 1) * 64],
                                    Dv[0:64, gl, R + 2:R + 10, kj:kj + 64],
                                    start=False, stop=(kj == 2))
                        t0 = scr.tile([128, 512], BF16, tag="t0")
                        for par in (0, 64):
                            nc.vector.tensor_scalar(
                                t0[par:par + 64, :],
                                ps[par:par + 64, (par * 8):(par * 8) + 512],
                                bc[par:par + 64, 1:2], 0.0,
                                op0=ALU.add, op1=ALU.max)
                        t0v = t0.rearrange("p (r c w) -> p r c w", r=8, c=32)
                        th = scr.tile([128, 256], BF16, tag="th")
                        thv = th.rearrange("p (r c) -> p r c", r=8)
                        nc.vector.tensor_tensor(
                            thv[:, :, :], t0v[:, :, :, 0], t0v[:, :, :, 1],
                            op=ALU.max)
                        PR = 4 * s
                        thv2 = th.rearrange("p (r w c) -> p r w c", r=4, w=2)
                        nc.vector.tensor_tensor(
                            D3v[0:64, ge, PR + 1:PR + 5, 1:33],
                            thv2[0:64, :, 0, :], thv2[0:64, :, 1, :],
                            op=ALU.max)
                        nc.vector.tensor_tensor(
                            D3v[64:128, go, PR:PR + 4, 1:33],
                            thv2[64:128, :, 0, :], thv2[64:128, :, 1, :],
                            op=ALU.max)
                    nc.sync.dma_start(
                        out=D3[0:64, go * SD3 + 34:go * SD3 + SD3],
                        in_=D3[64:128, go * SD3:go * SD3 + SD3 - 34])
                    nc.sync.dma_start(
                        out=D3[64:128, ge * SD3:ge * SD3 + SD3 - 34],
                        in_=D3[0:64, ge * SD3 + 34:ge * SD3 + SD3])

                # ---- L3 ----
                for gl in range(G):
                    ps = psum.tile([128, 1024], F32, tag="mm")
                    for kj in range(3):
                        for h in range(2):
                            nc.tensor.matmul(
                                ps[0:128, h * 512:(h + 1) * 512],
                                wst3[:, kj * 128:(kj + 1) * 128],
                                D3v[0:128, gl, 16 * h:16 * h + 16, kj:kj + 32],
                                start=(kj == 0), stop=False)
                    for kj in range(3):
                        for h in range(2):
                            nc.tensor.matmul(
                                ps[0:128, h * 512:(h + 1) * 512],
                                ws3[:, kj * 128:(kj + 1) * 128],
                                D3v[0:64, gl, 16 * h + 2:16 * h + 18, kj:kj + 32],
                                start=False, stop=(kj == 2))
                    t3 = scr.tile([128, 1024], BF16, tag="t3")
                    nc.scalar.activation(
                        t3[:, :], ps[:, :], AF.Relu, bias=bc[:, 2:3])
                    t3v = t3.rearrange("p (r c w) -> p r c w", r=32, c=16)
                    th3 = scr.tile([128, 512], BF16, tag="th3")
                    th3v = th3.rearrange("p (r c) -> p r c", r=32)
                    nc.vector.tensor_tensor(
                        th3v[:, :, :], t3v[:, :, :, 0], t3v[:, :, :, 1],
                        op=ALU.max)
                    th3v2 = th3.rearrange("p (r w c) -> p r w c", r=16, w=2)
                    nc.vector.tensor_tensor(
                        X4v[0:128, gl, 1:17, 1:17],
                        th3v2[:, :, 0, :], th3v2[:, :, 1, :], op=ALU.max)

                # ---- L4 (pairs share each weight via 2 live psum tiles) ----
                for f in range(2):
                    pss = [psum.tile([128, 1024], F32, tag="mm",
                                     name=f"ps4_{f}_{i}")
                           for i in range(G // 2)]
                    for ki in range(3):
                        for kj in range(3):
                            m = (ki * 3 + kj) * 2 + f
                            for q in range(G // 2):
                                ge = 2 * q
                                nc.tensor.matmul(
                                    pss[q][0:128, 0:512],
                                    w4[:, m * 128:(m + 1) * 128],
                                    X4v[0:128, ge:ge + 2, ki:ki + 16,
                                        kj:kj + 16],
                                    start=(ki == 0 and kj == 0),
                                    stop=(ki == 2 and kj == 2))
                    zt = zlo if f == 0 else zhi
                    for q in range(G // 2):
                        l4o = scr.tile([128, 512], BF16, tag="l4o")
                        for u in range(2):
                            img = gi * G + 2 * q + u
                            nc.scalar.activation(
                                l4o[:, u * 256:(u + 1) * 256],
                                pss[q][:, u * 256:(u + 1) * 256],
                                AF.Relu, bias=bc[:, 3 + f:4 + f],
                                accum_out=zt[:, img:img + 1])

            # one-group software pipeline: PE never waits on the evict/pool
            # chain of the group it just produced.
            for gi in range(NG):
                stage1(gi)
                if gi > 0:
                    stage234(gi - 1)
            stage234(NG - 1)

            # ---- FC: 256 -> 10 (fp32) ----
            fps = psum.tile([128, 1024], F32, tag="mm", name="fcps")[0:10, 0:NI]
            nc.tensor.matmul(fps[:, :], wfc[:, 0:10], zlo[:, :],
                             start=True, stop=False)
            nc.tensor.matmul(fps[:, :], wfc[:, 10:20], zhi[:, :],
                             start=False, stop=True)
            nc.scalar.activation(outb[:, :], fps[:, :], AF.Identity,
                                 bias=bfc[:, 0:1])
            nc.sync.dma_start(out=outd[:, :], in_=outb[:, :])

    if split_waits:
        _split_multi_waits(nc)
    return nc


# ---------------------------------------------------------------------------
# Host-side prep
# ---------------------------------------------------------------------------

def _fold_bn(w, b, g, be, m, v):
    s = (np.asarray(g, np.float64) / np.sqrt(np.asarray(v, np.float64) + 1e-5))
    wf = np.asarray(w, np.float64) * s[:, None, None, None]
    bf = (np.asarray(b, np.float64) - np.asarray(m, np.float64)) * s \
        + np.asarray(be, np.float64)
    return wf, bf


def _prep_core_inputs(x_shard, p, e):
    w1f, b1f = _fold_bn(p["c1w"][e], p["c1b"][e], p["bn1g"][e], p["bn1b"][e],
                        p["bn1m"][e], p["bn1v"][e])
    w2f, b2f = _fold_bn(p["c2w"][e], p["c2b"][e], p["bn2g"][e], p["bn2b"][e],
                        p["bn2m"][e], p["bn2v"][e])
    w3f, b3f = _fold_bn(p["c3w"][e], p["c3b"][e], p["bn3g"][e], p["bn3b"][e],
                        p["bn3m"][e], p["bn3v"][e])
    w4f, b4f = _fold_bn(p["c4w"][e], p["c4b"][e], p["bn4g"][e], p["bn4b"][e],
                        p["bn4m"][e], p["bn4v"][e])

    xp = np.pad(x_shard, ((0, 0), (0, 0), (1, 1), (1, 1)))
    arr = np.empty((27, NI, 64, 64), np.float32)
    for ki in range(3):
        for kj in range(3):
            for c in range(3):
                arr[(ki * 3 + kj) * 3 + c] = xp[:, c, ki:ki + 64, kj:kj + 64]
    b27 = np.zeros((128, NI, 16, 64), np.float32)
    for b in range(4):
        b27[32 * b:32 * b + 27] = arr[:, :, 16 * b:16 * b + 16, :]
    b27 = b27.reshape(128, NI * 1024).astype(_BF16)

    w1s = np.zeros((128, 64), np.float64)
    w1k = w1f.transpose(2, 3, 1, 0).reshape(27, 64)
    for b in range(4):
        w1s[32 * b:32 * b + 27] = w1k

    wst2 = np.zeros((128, 192), np.float64)
    ws2 = np.zeros((64, 192), np.float64)
    for kj in range(3):
        wst2[0:64, kj * 64:(kj + 1) * 64] = w2f[:, :, 0, kj].T
        wst2[64:128, kj * 64:(kj + 1) * 64] = w2f[:, :, 1, kj].T
        ws2[:, kj * 64:(kj + 1) * 64] = w2f[:, :, 2, kj].T

    wst3 = np.zeros((128, 384), np.float64)
    ws3 = np.zeros((64, 384), np.float64)
    for kj in range(3):
        wst3[0:64, kj * 128:(kj + 1) * 128] = w3f[:, :, 0, kj].T
        wst3[64:128, kj * 128:(kj + 1) * 128] = w3f[:, :, 1, kj].T
        ws3[:, kj * 128:(kj + 1) * 128] = w3f[:, :, 2, kj].T

    w4 = np.zeros((128, 18 * 128), np.float64)
    for ki in range(3):
        for kj in range(3):
            for f in range(2):
                m = (ki * 3 + kj) * 2 + f
                w4[:, m * 128:(m + 1) * 128] = \
                    w4f[128 * f:128 * (f + 1), :, ki, kj].T

    wf = (np.asarray(p["cw"][e], np.float64)
          @ np.asarray(p["pw"][e], np.float64)) / 256.0
    wfc = np.zeros((128, 20), np.float64)
    wfc[:, 0:10] = wf[:, 0:128].T
    wfc[:, 10:20] = wf[:, 128:256].T
    bfc = (np.asarray(p["cw"][e], np.float64)
           @ np.asarray(p["pb"][e], np.float64)
           + np.asarray(p["cb"][e], np.float64)).reshape(10, 1)

    bconv = np.zeros((128, 8), np.float64)
    bconv[0:64, 0] = b1f
    bconv[64:128, 0] = b1f
    bconv[0:64, 1] = b2f
    bconv[64:128, 1] = b2f
    bconv[:, 2] = b3f
    bconv[:, 3] = b4f[0:128]
    bconv[:, 4] = b4f[128:256]

    return {
        "b27": b27,
        "w1s": w1s.astype(_BF16), "wst2": wst2.astype(_BF16),
        "ws2": ws2.astype(_BF16), "wst3": wst3.astype(_BF16),
        "ws3": ws3.astype(_BF16), "w4": w4.astype(_BF16),
        "wfc": wfc.astype(np.float32), "bconv": bconv.astype(np.float32),
        "bfc": bfc.astype(np.float32),
    }


# ---------------------------------------------------------------------------
# Host-side scorer + waterfall routing (faithful to the reference)
# ---------------------------------------------------------------------------

def _scores_noisy(x, sw, sb, slw, slb):
    # scorer per expert: conv1x1 -> relu -> global avg pool -> linear(8->1)
    scores = np.empty((B, E), np.float32)
    xf = np.asarray(x, np.float32)
    for e in range(E):
        w = np.asarray(sw[e], np.float32)[:, :, 0, 0]          # [8,3]
        h = np.einsum("bchw,oc->bohw", xf, w, optimize=True) \
            + np.asarray(sb[e], np.float32)[None, :, None, None]
        h = np.maximum(h, 0.0)
        hm = h.mean(axis=(2, 3), dtype=np.float32)             # [B,8]
        scores[:, e] = hm @ np.asarray(slw[e], np.float32)[0] \
            + np.asarray(slb[e], np.float32)[0]

    # Match the reference bit-for-bit: same jax calls, default device/PRNG
    # (this env defaults to the rbg PRNG, which is backend-dependent).
    import jax
    import jax.numpy as jnp
    u = jax.random.uniform(
        jax.random.key(42), (B, E), jnp.float32, 1e-7, 1.0 - 1e-7)
    gumbel = -jnp.log(-jnp.log(u))
    return np.asarray((jnp.asarray(scores) + gumbel) / TEMP)


def _route(scores_noisy, cap_C):
    s = np.asarray(scores_noisy, np.float64)
    Bn, En = s.shape
    owner = -np.ones(Bn, np.int64)
    cap = np.zeros(En, np.int64)
    remaining = np.arange(Bn)
    it = 0
    while remaining.size > 0:
        st = s[remaining].copy()
        deficit = np.clip(cap / float(cap_C), 0.0, 1.0)
        st = st * (1.0 - deficit)
        full = cap >= cap_C
        st[:, full] = -np.inf
        best = st.argmax(axis=1)
        taken = np.zeros(remaining.size, bool)
        quota = 2 ** it
        for e in range(En):
            want = np.nonzero(best == e)[0]
            if want.size == 0:
                continue
            space = min(cap_C - int(cap[e]), quota)
            if space <= 0:
                continue
            sel = want[:space]
            owner[remaining[sel]] = e
            cap[e] += sel.size
            taken[sel] = True
        remaining = remaining[~taken]
        it += 1
    idx = np.zeros((En, cap_C), np.int64)
    mask = np.zeros((En, cap_C), np.float32)
    for e in range(En):
        ids = np.nonzero(owner == e)[0]
        idx[e, :ids.size] = ids
        mask[e, :ids.size] = 1.0
    return idx, mask


# ---------------------------------------------------------------------------
# Entry point
# ---------------------------------------------------------------------------

_NC_CACHE = {}


def kernel(**inputs):
    from concourse.bass_utils import run_bass_kernel_spmd

    _install_drain_patch()

    p = {k: np.asarray(v) for k, v in inputs.items()}
    x = p["x"].astype(np.float32, copy=False)

    sn = _scores_noisy(x, p["sw"], p["sb"], p["slw"], p["slb"])
    idx, mask = _route(sn, CAP)

    if "nc" not in _NC_CACHE:
        _NC_CACHE["nc"] = _build_nc()
    nc = _NC_CACHE["nc"]

    in_maps = []
    for core in range(8):
        e = core // 2
        half = core % 2
        ids = idx[e, half * NI:(half + 1) * NI]
        in_maps.append(_prep_core_inputs(x[ids], p, e))

    res = run_bass_kernel_spmd(nc, in_maps, list(range(8)))
    outs = res.results

    out = np.zeros((B, NCLS), np.float32)
    for core in range(8):
        e = core // 2
        half = core % 2
        ids = idx[e, half * NI:(half + 1) * NI]
        m = mask[e, half * NI:(half + 1) * NI]
        np.add.at(out, ids, outs[core]["out"].T * m[:, None])
    return out
